# revision 49
# baseline (speedup 1.0000x reference)
"""Trainium2 Bass kernel for nn_Attention_9242769622327.

Math: the reference computes
    qkv = x @ W1.T ; q,k,v = split(qkv)
    score = softmax(k^T v / 4, axis=-1)            # rows sum to 1
    attn  = softmax(einsum('bhnk,bhkc->bhnk', q/4, score), axis=-1)
          = softmax(q/4 * sum_c score)             # sum_c score == 1
          = softmax(q/4)                           # k/v are mathematically dead
    out   = attn @ W2.T
so only the q-projection (first E rows of W1), a per-head (64-wide) softmax,
and the output projection are needed.

Distribution: pure data-parallel over the 32768 = B*S rows; each of the 8
cores handles 4096 rows with the full (transposed) weights. No collectives.

Both big GEMMs run in fp8 e4m3 with perf_mode=DoubleRow (2 fp8 MACs per PE
cell per cycle, K=256 per matmul).  fp8's ~2% rounding noise is kept out of
the output via two tricks:
  * mm1 noise enters pre-softmax and is damped 4x by the q/4 scale; weights
    are pre-scaled by 64 (W1q values ~N(0, 1/32^2) would land in e4m3's
    subnormal range) and the exp() activation scale absorbs the 1/64.
  * mm2 operates on d = 64*attn - 1 instead of attn: per head the 64 attn
    values sum to 1, so attn = 1/64 + small delta and quantizing the delta
    is 4x less noisy than quantizing attn.  out = W2 @ attn is reconstructed
    as out[j,m] = P[j,m]/4096 + S_j/64 with P = (64*W2)_fp8 @ d_fp8 and
    S_j = sum_n W2[j,n] computed exactly on the host (folded into the
    PSUM->SBUF copy as a per-partition bias).

On-chip layout is fully transposed (features on partitions, rows on the free
dim) so no on-chip transposes are needed anywhere:
    q64[n,m] = sum_k 64*W1qT[k,n] * xT[k,m]       (PE, fp8 DoubleRow)
    u        = exp(q64/256)                       (ACT, PSUM->SBUF fp16)
    u8       = fp8(u)                             (DVE copy; its 2% noise
                                                   averages down over the 64
                                                   positive head-sum terms)
    s[g,m]   = sum_{n in head g} u8[n,m] / 64     (PE, fp8 DoubleRow selector)
    rcp      = 64/s                               (DVE reciprocal_approx_fast)
    rcp8     = hi/lo fp8 pair of rcp              (ACT copy + DVE residual)
    rb64     = selt^T @ rcp8                      (PE, fp8 DoubleRow selector^T)
    at64     = u * rb64                           (DVE, fp16 = 64*attn)
    d8       = at64 - 1                           (DVE, fp8)
    P[j,m]   = sum_n (64*W2T)[n,j] * d8[n,m]      (PE, fp8 DoubleRow)
    outT     = P/4096 + S_j/64                    (ACT Identity w/ bias vec)

Stripes are software-pipelined: [8 rb(prev)][32 mm1(ms)][4 sel(ms)]
[32 mm2(prev)] as contiguous matmul blocks on the PE, with exp/reciprocal/
normalize/quantize hidden underneath on ACT/DVE.  The last stripe's norm
block is emitted right after its sel block so its d8 chain hides under
mm2 of the previous stripe, shortening the exposed epilogue.  PSUM is at
exactly 8 banks (q:3, s:1, rb:2, o:2).
"""

import sys

sys.path.insert(0, "/opt/trn_rl_repo")

import numpy as np
import ml_dtypes

import concourse.bass as bass
import concourse.bacc as bacc
import concourse.tile as tile
from concourse import mybir
from concourse.bass_utils import run_bass_kernel_spmd

F16 = mybir.dt.float16
F8 = mybir.dt.float8e4
F32 = mybir.dt.float32
AF = mybir.ActivationFunctionType
DR = mybir.MatmulPerfMode.DoubleRow

N_CORES = 8
B, S, E = 4, 8192, 1024
HEADS, HEAD_DIM = 16, 64
M_TOTAL = B * S                # 32768
M_CORE = M_TOTAL // N_CORES    # 4096 rows per core
KC2 = E // 256                 # 4 double-row contraction chunks
NC_ = E // 128                 # 8 feature chunks
WSCALE = 64.0                  # host pre-scale on W1q and W2

# m-stripes: 512 wide (one PSUM bank of fp32 per tile)
STRIPES = [(m0, 512) for m0 in range(0, M_CORE, 512)]

_NF16 = np.float16
_NF8 = ml_dtypes.float8_e4m3   # == TRN FP8_EXP4 (max normal 240, has inf)


def build_nc() -> bass.Bass:
    nc = bacc.Bacc("TRN2", debug=False)

    xt = nc.dram_tensor("xt", [E, M_CORE], F8, kind="ExternalInput")
    w1t = nc.dram_tensor("w1t", [E, E], F8, kind="ExternalInput")
    w2t = nc.dram_tensor("w2t", [E, E], F8, kind="ExternalInput")
    sel8 = nc.dram_tensor("sel8", [128, KC2 * 2 * HEADS], F8, kind="ExternalInput")
    selt = nc.dram_tensor("selt", [128, 2 * NC_ * 128], F8, kind="ExternalInput")
    bias = nc.dram_tensor("bias", [128, NC_], F32, kind="ExternalInput")
    outT = nc.dram_tensor("outT", [E, M_CORE], F16, kind="ExternalOutput")

    # k = c*256 + i*128 + p  (DoubleRow pair plane i, partition p)
    xt_v = xt[:, :].rearrange("(c i p) m -> p c i m", p=128, i=2)
    w1_v = w1t[:, :].rearrange("(c i p) n -> p c i n", p=128, i=2)
    w2_v = w2t[:, :].rearrange("(c i p) j -> p c i j", p=128, i=2)

    with tile.TileContext(nc) as tc:
        with (
            tc.tile_pool(name="weights", bufs=1) as wpool,
            tc.tile_pool(name="xt", bufs=4) as xpool,
            tc.tile_pool(name="u", bufs=24) as upool,
            tc.tile_pool(name="u8", bufs=4) as u8pool,
            tc.tile_pool(name="at", bufs=24) as apool,
            tc.tile_pool(name="d8", bufs=8) as dpool,
            tc.tile_pool(name="small", bufs=4) as spool,
            tc.tile_pool(name="ostage", bufs=12) as opool,
            tc.tile_pool(name="ps_q", bufs=3, space="PSUM") as psq,
            tc.tile_pool(name="ps_s", bufs=1, space="PSUM") as pss,
            tc.tile_pool(name="ps_rb", bufs=2, space="PSUM") as psrb,
            tc.tile_pool(name="ps_o", bufs=2, space="PSUM") as pso,
        ):
            # Warm the PE's HAM clock gate with short throwaway matmuls while
            # the first weight/x DMAs are in flight (HAM needs ~3.4us of
            # sustained PE activity to unthrottle 1.2 -> 2.4 GHz).  N=256 so
            # the block ends about when the first real operands land instead
            # of queueing 8us of fake work ahead of them.
            warm_sb = wpool.tile([128, 256], F16, name="warm_sb")
            nc.gpsimd.memset(warm_sb[:], 0.0)
            neg1 = wpool.tile([128, 1], F32, name="neg1")
            nc.gpsimd.memset(neg1[:], -1.0)
            warm_ps = psq.tile([128, 512], F32, tag="q", name="warm_ps")
            for _ in range(10):
                nc.tensor.matmul(
                    warm_ps[:, 0:256], warm_sb[:, 0:128], warm_sb[:],
                    start=True, stop=True,
                )

            # Per-chunk weight tiles so the first matmuls only wait on the
            # chunks they read, not the whole 2MB of weights.  Load order:
            # w1 + stripe-0 x chunks (needed by stripe 0's mm1), sel8, then
            # w2 + selt + bias (not needed until ~10us in).
            w1_k = []
            xt0 = []
            for c in range(KC2):
                t = wpool.tile([128, 2, E], F8, tag=f"w1_{c}", name=f"w1k{c}")
                # column-halved so mm1's first ci blocks wait only on the
                # weights they read (Tile deps are region-precise)
                nc.sync.dma_start(t[:, :, 0:512], w1_v[:, c, :, 0:512])
                nc.sync.dma_start(t[:, :, 512:E], w1_v[:, c, :, 512:E])
                w1_k.append(t)
                tx = xpool.tile([128, 2, 512], F8, tag=f"xt_{c}", name=f"xt0_{c}")
                nc.sync.dma_start(tx[:], xt_v[:, c, :, 0:512])
                xt0.append(tx)
            sel8_t = wpool.tile([128, KC2, 2, HEADS], F8, name="sel8_t")
            nc.sync.dma_start(
                sel8_t[:],
                sel8[:, :].rearrange("p (c i g) -> p c i g", i=2, g=HEADS),
            )

            # Prefetch x for stripes 1-2 BEFORE the w2/selt/bias block: w2 is
            # not needed until ~18us in, but x1 gates stripe 1's mm1 at ~17us
            # and would otherwise queue behind 1MB of w2.
            xpre = {}
            for si in (1, 2):
                m0, mw = STRIPES[si]
                tiles = []
                for c in range(KC2):
                    t = xpool.tile([128, 2, 512], F8, tag=f"xt_{c}", name=f"xt{si}_{c}")
                    nc.sync.dma_start(t[:, :, 0:mw], xt_v[:, c, :, m0:m0 + mw])
                    tiles.append(t)
                xpre[si] = tiles

            w2_k = []
            for c in range(KC2):
                t = wpool.tile([128, 2, E], F8, tag=f"w2_{c}", name=f"w2k{c}")
                nc.sync.dma_start(t[:], w2_v[:, c, :, :])
                w2_k.append(t)
            selt_t = wpool.tile([128, 2, NC_, 128], F8, name="selt_t")
            nc.sync.dma_start(
                selt_t[:], selt[:, :].rearrange("p (i c q) -> p i c q", i=2, q=128)
            )
            bias_t = wpool.tile([128, NC_], F32, name="bias_t")
            nc.sync.dma_start(bias_t[:], bias[:, :])

            # Software pipeline over stripes: stripe ms runs
            #   [rb(prev)][mm1(ms)][sel(ms)][mm2(prev)]
            # on the PE; the rb->at64->d8 chain (DVE+GPSIMD) for the previous
            # stripe hides under mm1(ms), and exp(ms) (ACT) lands just after
            # mm1(ms).
            prev = None  # (u_tiles, rcp_t, m0, mw) of previous stripe

            def emit_norm(pu, prcp, mw, subs_on_act=False):
                """rb broadcast matmuls (PE, DoubleRow block) + normalize
                (DVE) / quantize (DVE, or ACT for the last stripe where DVE
                is the backlogged engine and ACT is idle)."""
                d_tiles = [
                    dpool.tile([128, 2, 512], F8, tag=f"d8_{c}", name=f"d8_{c}")
                    for c in range(KC2)
                ]
                for ci in range(NC_):
                    rb_ps = psrb.tile([128, 512], F32, tag="rb", name="rb_ps")
                    nc.tensor.matmul(
                        rb_ps[:, 0:mw], selt_t[:, :, ci, :], prcp[:, :, 0:mw],
                        start=True, stop=True, perf_mode=DR,
                    )
                    at_t = apool.tile([128, 512], F16, tag="at", name="at_t")
                    nc.vector.tensor_mul(
                        at_t[:, 0:mw], pu[ci][:, 0:mw], rb_ps[:, 0:mw]
                    )
                    if subs_on_act:
                        nc.scalar.activation(
                            d_tiles[ci // 2][:, ci % 2, 0:mw], at_t[:, 0:mw],
                            AF.Identity, bias=neg1[:],
                        )
                    else:
                        nc.vector.tensor_scalar_sub(
                            d_tiles[ci // 2][:, ci % 2, 0:mw], at_t[:, 0:mw], 1.0
                        )
                return d_tiles

            def emit_tail(d_tiles, m0, mw, last=False, j_range=None):
                """Emit mm2 + store for a finished stripe (d8 tiles ready)."""
                for j in (j_range if j_range is not None else range(NC_)):
                    o_ps = pso.tile([128, 512], F32, tag="o", name="o_ps")
                    for c in range(KC2):
                        nc.tensor.matmul(
                            o_ps[:, 0:mw],
                            w2_k[c][:, :, j * 128:(j + 1) * 128],
                            d_tiles[c][:, :, 0:mw],
                            start=(c == 0),
                            stop=(c == KC2 - 1),
                            perf_mode=DR,
                        )
                    o_t = opool.tile([128, 512], F16, tag="ost", name="o_t")
                    # the very last output block is staged out in two halves
                    # so its store overlaps the ACT copy of the second half
                    halves = (
                        [(0, mw // 2), (mw // 2, mw)]
                        if (last and j == NC_ - 1) else [(0, mw)]
                    )
                    for lo, hi in halves:
                        nc.scalar.activation(
                            o_t[:, lo:hi], o_ps[:, lo:hi], AF.Identity,
                            bias=bias_t[:, j:j + 1], scale=1.0 / (WSCALE * WSCALE),
                        )
                        nc.sync.dma_start(
                            outT[j * 128:(j + 1) * 128, m0 + lo:m0 + hi],
                            o_t[:, lo:hi],
                        )

            for si, (m0, mw) in enumerate(STRIPES):
                if si == 0:
                    xt_k = xt0
                elif si in xpre:
                    xt_k = xpre[si]
                else:
                    xt_k = []
                    for c in range(KC2):
                        t = xpool.tile(
                            [128, 2, 512], F8, tag=f"xt_{c}", name=f"xt{si}_{c}"
                        )
                        nc.sync.dma_start(
                            t[:, :, 0:mw], xt_v[:, c, :, m0:m0 + mw]
                        )
                        xt_k.append(t)

                # ---- previous stripe normalization (rb block first: its d8
                # chain then hides under mm1 of this stripe) ----
                d_tiles = emit_norm(prev[0], prev[1], prev[3]) if prev else None
                last = si == len(STRIPES) - 1

                # ---- mm1: q-projection, contiguous 32-MM DoubleRow block ----
                u_tiles = []
                u8_tiles = [
                    u8pool.tile([128, 2, 512], F8, tag=f"u8_{c}", name=f"u8_{c}")
                    for c in range(KC2)
                ]

                def emit_mm1(ci_range):
                    for ci in ci_range:
                        # Borrow idle PSUM banks during mm1 to deepen the q
                        # ring to 5: the sel-sum bank for ci=3 and an output
                        # bank for ci=7 (its previous reader, ident(ms-2),
                        # completed two PE blocks ago).
                        if ci == 3:
                            q_ps = pss.tile([128, 512], F32, tag="s", name="q_ps_s")
                        elif ci == 7:
                            q_ps = pso.tile([128, 512], F32, tag="o", name="q_ps_o")
                        else:
                            q_ps = psq.tile([128, 512], F32, tag="q", name="q_ps")
                        for c in range(KC2):
                            nc.tensor.matmul(
                                q_ps[:, 0:mw],
                                w1_k[c][:, :, ci * 128:(ci + 1) * 128],
                                xt_k[c][:, :, 0:mw],
                                start=(c == 0),
                                stop=(c == KC2 - 1),
                                perf_mode=DR,
                            )
                        u_t = upool.tile([128, 512], F16, tag="u", name="u_t")
                        nc.scalar.activation(
                            u_t[:, 0:mw], q_ps[:, 0:mw], AF.Exp, scale=0.25 / WSCALE
                        )
                        nc.vector.tensor_scalar_mul(
                            u8_tiles[ci // 2][:, ci % 2, 0:mw], u_t[:, 0:mw], 1.0
                        )
                        u_tiles.append(u_t)

                if last and d_tiles is not None:
                    # Last stripe: there is no next stripe to hide this
                    # stripe's tail under, so pull the PREVIOUS stripe's mm2
                    # forward — half into the middle of mm1 (its output
                    # copies unblock while mm1's second half runs) and half
                    # after sel (absorbing the rb reciprocal-chain latency) —
                    # leaving only sel+rb+mm2(last) exposed after the final
                    # mm1 block.
                    emit_mm1(range(0, 4))
                    emit_tail(d_tiles, prev[2], prev[3], j_range=range(0, 4))
                    emit_mm1(range(4, NC_))
                else:
                    emit_mm1(range(NC_))

                # ---- head sums (4-MM fp8 DoubleRow block) + reciprocal ----
                s_ps = pss.tile([128, 512], F32, tag="s", name="s_ps")
                for c in range(KC2):
                    nc.tensor.matmul(
                        s_ps[0:HEADS, 0:mw],
                        sel8_t[:, c, :, :],
                        u8_tiles[c][:, :, 0:mw],
                        start=(c == 0),
                        stop=(c == KC2 - 1),
                        perf_mode=DR,
                    )
                # sel8 entries are 1/64, so s_ps = s/64 and rcp32 = 64/s
                rcp32 = spool.tile([HEADS, 512], F32, tag="rcp32", name="rcp32")
                nc.vector.reciprocal_approx_fast(rcp32[:, 0:mw], s_ps[0:HEADS, 0:mw])
                # 64/s as an fp8 hi/lo pair (so the rb broadcast matmul can be
                # DoubleRow like its neighbors - no fp16<->fp8 weight-buffer
                # mode switches on the PE).  Rows 16+ zeroed on GpSimd.
                rcp_t = spool.tile([128, 2, 512], F8, tag="rcp", name="rcp_t")
                nc.gpsimd.memset(rcp_t[:, :, 0:mw], 0.0)
                nc.scalar.activation(
                    rcp_t[0:HEADS, 0, 0:mw], rcp32[:, 0:mw], AF.Copy
                )
                nc.vector.scalar_tensor_tensor(
                    rcp_t[0:HEADS, 1, 0:mw],
                    rcp_t[0:HEADS, 0, 0:mw],
                    -1.0,
                    rcp32[:, 0:mw],
                    op0=mybir.AluOpType.mult,
                    op1=mybir.AluOpType.add,
                )

                if last:
                    # second half of the previous stripe's tail fills the
                    # rb(last) reciprocal-chain latency; then the last
                    # stripe's own norm block (rb + d8 chain, subs on the
                    # now-idle ACT engine).
                    if d_tiles is not None:
                        emit_tail(d_tiles, prev[2], prev[3], j_range=range(4, NC_))
                    d_last = emit_norm(u_tiles, rcp_t, mw, subs_on_act=True)
                else:
                    # ---- previous stripe output projection ----
                    if d_tiles is not None:
                        emit_tail(d_tiles, prev[2], prev[3])
                prev = (u_tiles, rcp_t, m0, mw)

            # epilogue: last stripe's output projection
            emit_tail(d_last, prev[2], prev[3], last=True)
    nc.compile()
    return nc


_NC_CACHE = None
LAST_RESULT = None


def _ensure_ntff_hook():
    """bass_utils' axon trace path needs antenv.axon_hooks, which this
    container's antenv lacks. Provide it + register the ctypes NTFF hook."""
    import types

    try:
        from antenv.axon_hooks import get_axon_ntff_profile_hook  # noqa: F401
        return True
    except ImportError:
        pass
    try:
        import antenv
        from trn_agent_boot.trn_boot import _ntff_profile_via_ctypes

        m = types.ModuleType("antenv.axon_hooks")
        state = {"hook": None}
        m.set_axon_ntff_profile_hook = lambda h: state.__setitem__("hook", h)
        m.get_axon_ntff_profile_hook = lambda: state["hook"]
        sys.modules["antenv.axon_hooks"] = m
        antenv.axon_hooks = m
        m.set_axon_ntff_profile_hook(
            _ntff_profile_via_ctypes("/opt/axon/libaxon_pjrt.so")
        )
        return True
    except Exception as e:  # pragma: no cover
        print(f"ntff hook injection failed: {e}")
        return False


def _selectors():
    # head index of global feature n is n // 64.
    # sel8: DoubleRow selector for the head-sum, paired like u8/d8 tiles:
    #   plane (c, i) covers feature chunk ci = 2c+i, i.e. heads 2*ci (parts
    #   0..63) and 2*ci+1 (parts 64..127).
    # Entries are 1/64 so the head-sum comes out pre-scaled: s_ps = s/64,
    # making reciprocal_approx_fast produce 64/s directly.
    sel8 = np.zeros((128, KC2, 2, HEADS), np.float32)
    for c in range(KC2):
        for i in range(2):
            ci = 2 * c + i
            sel8[:64, c, i, 2 * ci] = 1.0 / 64.0
            sel8[64:, c, i, 2 * ci + 1] = 1.0 / 64.0
    # selt: transposed selector for the rcp broadcast matmul.  Two identical
    # 0/1 planes (DoubleRow pairs): plane 0 multiplies rcp_hi, plane 1 the
    # fp8 residual rcp_lo; their PSUM sum reconstructs 64/s to ~1e-3.
    selt = np.zeros((128, 2, NC_, 128), np.float32)
    for ci in range(NC_):
        for i in range(2):
            selt[2 * ci, i, ci, :64] = 1.0
            selt[2 * ci + 1, i, ci, 64:] = 1.0
    return (
        np.ascontiguousarray(sel8.reshape(128, KC2 * 2 * HEADS)).astype(_NF8),
        np.ascontiguousarray(selt.reshape(128, 2 * NC_ * 128)).astype(_NF8),
    )


def kernel(x, W1, W2, heads, trace=False):
    global _NC_CACHE, LAST_RESULT
    x = np.asarray(x, dtype=np.float32)
    W1 = np.asarray(W1, dtype=np.float32)
    W2 = np.asarray(W2, dtype=np.float32)

    X = x.reshape(M_TOTAL, E)
    X8T = X.astype(_NF8).T  # [E, M_TOTAL] view
    w1t = np.ascontiguousarray(W1[:E, :].T * WSCALE).astype(_NF8)  # 64*W1q[n,k]^T
    w2t = np.ascontiguousarray(W2.T * WSCALE).astype(_NF8)         # 64*W2[j,n]^T
    sel8, selt = _selectors()
    # bias[p, j] = S_{j*128+p} / 64 with S_j = sum_n W2[j, n] (exact fp32)
    bias = np.ascontiguousarray(
        (W2.sum(axis=1) / WSCALE).reshape(NC_, 128).T
    ).astype(np.float32)

    in_maps = []
    for c in range(N_CORES):
        xt_c = np.ascontiguousarray(X8T[:, c * M_CORE:(c + 1) * M_CORE])
        in_maps.append(
            {"xt": xt_c, "w1t": w1t, "w2t": w2t, "sel8": sel8, "selt": selt,
             "bias": bias}
        )

    if _NC_CACHE is None:
        _NC_CACHE = build_nc()

    if trace:
        trace = _ensure_ntff_hook()

    res = run_bass_kernel_spmd(_NC_CACHE, in_maps, list(range(N_CORES)), trace=trace)
    LAST_RESULT = res

    OT = np.concatenate(
        [np.asarray(res.results[c]["outT"]).astype(np.float32) for c in range(N_CORES)],
        axis=1,
    )
    return np.ascontiguousarray(OT.T).reshape(B, S, E)


# revision 51
# speedup vs baseline: 1.0148x; 1.0148x over previous
"""Trainium2 Bass kernel for nn_Attention_9242769622327.

Math: the reference computes
    qkv = x @ W1.T ; q,k,v = split(qkv)
    score = softmax(k^T v / 4, axis=-1)            # rows sum to 1
    attn  = softmax(einsum('bhnk,bhkc->bhnk', q/4, score), axis=-1)
          = softmax(q/4 * sum_c score)             # sum_c score == 1
          = softmax(q/4)                           # k/v are mathematically dead
    out   = attn @ W2.T
so only the q-projection (first E rows of W1), a per-head (64-wide) softmax,
and the output projection are needed.

Distribution: pure data-parallel over the 32768 = B*S rows; each of the 8
cores handles 4096 rows with the full (transposed) weights. No collectives.

Both big GEMMs run in fp8 e4m3 with perf_mode=DoubleRow (2 fp8 MACs per PE
cell per cycle, K=256 per matmul).  fp8's ~2% rounding noise is kept out of
the output via two tricks:
  * mm1 noise enters pre-softmax and is damped 4x by the q/4 scale; weights
    are pre-scaled by 64 (W1q values ~N(0, 1/32^2) would land in e4m3's
    subnormal range) and the exp() activation scale absorbs the 1/64.
  * mm2 operates on d = 64*attn - 1 instead of attn: per head the 64 attn
    values sum to 1, so attn = 1/64 + small delta and quantizing the delta
    is 4x less noisy than quantizing attn.  out = W2 @ attn is reconstructed
    as out[j,m] = P[j,m]/4096 + S_j/64 with P = (64*W2)_fp8 @ d_fp8 and
    S_j = sum_n W2[j,n] computed exactly on the host (folded into the
    PSUM->SBUF copy as a per-partition bias).

On-chip layout is fully transposed (features on partitions, rows on the free
dim) so no on-chip transposes are needed anywhere:
    q64[n,m] = sum_k 64*W1qT[k,n] * xT[k,m]       (PE, fp8 DoubleRow)
    u        = exp(q64/256)                       (ACT, PSUM->SBUF fp16)
    u8       = fp8(u)                             (DVE copy; its 2% noise
                                                   averages down over the 64
                                                   positive head-sum terms)
    s[g,m]   = sum_{n in head g} u8[n,m] / 64     (PE, fp8 DoubleRow selector)
    rcp      = 64/s                               (DVE reciprocal_approx_fast)
    rcp8     = hi/lo fp8 pair of rcp              (ACT copy + DVE residual)
    rb64     = selt^T @ rcp8                      (PE, fp8 DoubleRow selector^T)
    at64     = u * rb64                           (DVE, fp16 = 64*attn)
    d8       = at64 - 1                           (DVE, fp8)
    P[j,m]   = sum_n (64*W2T)[n,j] * d8[n,m]      (PE, fp8 DoubleRow)
    outT     = P/4096 + S_j/64                    (ACT Identity w/ bias vec)

Stripes are software-pipelined: [8 rb(prev)][32 mm1(ms)][4 sel(ms)]
[32 mm2(prev)] as contiguous matmul blocks on the PE, with exp/reciprocal/
normalize/quantize hidden underneath on ACT/DVE.  The last stripe's norm
block is emitted right after its sel block so its d8 chain hides under
mm2 of the previous stripe, shortening the exposed epilogue.  PSUM is at
exactly 8 banks (q:3, s:1, rb:2, o:2).
"""

import sys

sys.path.insert(0, "/opt/trn_rl_repo")

import numpy as np
import ml_dtypes

import concourse.bass as bass
import concourse.bacc as bacc
import concourse.tile as tile
from concourse import mybir
from concourse.bass_utils import run_bass_kernel_spmd

F16 = mybir.dt.float16
F8 = mybir.dt.float8e4
F32 = mybir.dt.float32
AF = mybir.ActivationFunctionType
DR = mybir.MatmulPerfMode.DoubleRow

N_CORES = 8
B, S, E = 4, 8192, 1024
HEADS, HEAD_DIM = 16, 64
M_TOTAL = B * S                # 32768
M_CORE = M_TOTAL // N_CORES    # 4096 rows per core
KC2 = E // 256                 # 4 double-row contraction chunks
NC_ = E // 128                 # 8 feature chunks
WSCALE = 64.0                  # host pre-scale on W1q and W2

# m-stripes: 512 wide (one PSUM bank of fp32 per tile)
STRIPES = [(m0, 512) for m0 in range(0, M_CORE, 512)]

_NF16 = np.float16
_NF8 = ml_dtypes.float8_e4m3   # == TRN FP8_EXP4 (max normal 240, has inf)


def build_nc() -> bass.Bass:
    nc = bacc.Bacc("TRN2", debug=False)

    xt = nc.dram_tensor("xt", [E, M_CORE], F8, kind="ExternalInput")
    w1t = nc.dram_tensor("w1t", [E, E], F8, kind="ExternalInput")
    w2t = nc.dram_tensor("w2t", [E, E], F8, kind="ExternalInput")
    sel8 = nc.dram_tensor("sel8", [128, KC2 * 2 * HEADS], F8, kind="ExternalInput")
    selt = nc.dram_tensor("selt", [128, 2 * NC_ * 128], F8, kind="ExternalInput")
    bias = nc.dram_tensor("bias", [128, NC_], F32, kind="ExternalInput")
    outT = nc.dram_tensor("outT", [E, M_CORE], F16, kind="ExternalOutput")

    # k = c*256 + i*128 + p  (DoubleRow pair plane i, partition p)
    xt_v = xt[:, :].rearrange("(c i p) m -> p c i m", p=128, i=2)
    w1_v = w1t[:, :].rearrange("(c i p) n -> p c i n", p=128, i=2)
    w2_v = w2t[:, :].rearrange("(c i p) j -> p c i j", p=128, i=2)

    with tile.TileContext(nc) as tc:
        with (
            tc.tile_pool(name="weights", bufs=1) as wpool,
            tc.tile_pool(name="xt", bufs=4) as xpool,
            tc.tile_pool(name="u", bufs=24) as upool,
            tc.tile_pool(name="u8", bufs=4) as u8pool,
            tc.tile_pool(name="at", bufs=24) as apool,
            tc.tile_pool(name="d8", bufs=8) as dpool,
            tc.tile_pool(name="small", bufs=4) as spool,
            tc.tile_pool(name="ostage", bufs=12) as opool,
            tc.tile_pool(name="ps_q", bufs=3, space="PSUM") as psq,
            tc.tile_pool(name="ps_s", bufs=1, space="PSUM") as pss,
            tc.tile_pool(name="ps_rb", bufs=2, space="PSUM") as psrb,
            tc.tile_pool(name="ps_o", bufs=2, space="PSUM") as pso,
        ):
            # Warm the PE's HAM clock gate with short throwaway matmuls while
            # the first weight/x DMAs are in flight (HAM needs ~3.4us of
            # sustained PE activity to unthrottle 1.2 -> 2.4 GHz).  N=256 so
            # the block ends about when the first real operands land instead
            # of queueing 8us of fake work ahead of them.
            warm_sb = wpool.tile([128, 256], F16, name="warm_sb")
            nc.gpsimd.memset(warm_sb[:], 0.0)
            neg1 = wpool.tile([128, 1], F32, name="neg1")
            nc.gpsimd.memset(neg1[:], -1.0)
            warm_ps = psq.tile([128, 512], F32, tag="q", name="warm_ps")
            for _ in range(10):
                nc.tensor.matmul(
                    warm_ps[:, 0:256], warm_sb[:, 0:128], warm_sb[:],
                    start=True, stop=True,
                )

            # Per-chunk weight tiles so the first matmuls only wait on the
            # chunks they read, not the whole 2MB of weights.  Load order:
            # w1 + stripe-0 x chunks (needed by stripe 0's mm1), sel8, then
            # w2 + selt + bias (not needed until ~10us in).
            w1_k = []
            xt0 = []
            for c in range(KC2):
                t = wpool.tile([128, 2, E], F8, tag=f"w1_{c}", name=f"w1k{c}")
                nc.sync.dma_start(t[:], w1_v[:, c, :, :])
                w1_k.append(t)
                tx = xpool.tile([128, 2, 512], F8, tag=f"xt_{c}", name=f"xt0_{c}")
                nc.sync.dma_start(tx[:], xt_v[:, c, :, 0:512])
                xt0.append(tx)
            sel8_t = wpool.tile([128, KC2, 2, HEADS], F8, name="sel8_t")
            nc.sync.dma_start(
                sel8_t[:],
                sel8[:, :].rearrange("p (c i g) -> p c i g", i=2, g=HEADS),
            )

            # Prefetch x for stripes 1-2 BEFORE the w2/selt/bias block: w2 is
            # not needed until ~18us in, but x1 gates stripe 1's mm1 at ~17us
            # and would otherwise queue behind 1MB of w2.
            xpre = {}
            for si in (1, 2):
                m0, mw = STRIPES[si]
                tiles = []
                for c in range(KC2):
                    t = xpool.tile([128, 2, 512], F8, tag=f"xt_{c}", name=f"xt{si}_{c}")
                    nc.sync.dma_start(t[:, :, 0:mw], xt_v[:, c, :, m0:m0 + mw])
                    tiles.append(t)
                xpre[si] = tiles

            w2_k = []
            for c in range(KC2):
                t = wpool.tile([128, 2, E], F8, tag=f"w2_{c}", name=f"w2k{c}")
                nc.sync.dma_start(t[:], w2_v[:, c, :, :])
                w2_k.append(t)
            selt_t = wpool.tile([128, 2, NC_, 128], F8, name="selt_t")
            nc.sync.dma_start(
                selt_t[:], selt[:, :].rearrange("p (i c q) -> p i c q", i=2, q=128)
            )
            bias_t = wpool.tile([128, NC_], F32, name="bias_t")
            nc.sync.dma_start(bias_t[:], bias[:, :])

            # Software pipeline over stripes: stripe ms runs
            #   [rb(prev)][mm1(ms)][sel(ms)][mm2(prev)]
            # on the PE; the rb->at64->d8 chain (DVE+GPSIMD) for the previous
            # stripe hides under mm1(ms), and exp(ms) (ACT) lands just after
            # mm1(ms).
            prev = None  # (u_tiles, rcp_t, m0, mw) of previous stripe

            def emit_norm(pu, prcp, mw, subs_on_act=False):
                """rb broadcast matmuls (PE, DoubleRow block) + normalize
                (DVE) / quantize (DVE, or ACT for the last stripe where DVE
                is the backlogged engine and ACT is idle)."""
                d_tiles = [
                    dpool.tile([128, 2, 512], F8, tag=f"d8_{c}", name=f"d8_{c}")
                    for c in range(KC2)
                ]
                for ci in range(NC_):
                    rb_ps = psrb.tile([128, 512], F32, tag="rb", name="rb_ps")
                    nc.tensor.matmul(
                        rb_ps[:, 0:mw], selt_t[:, :, ci, :], prcp[:, :, 0:mw],
                        start=True, stop=True, perf_mode=DR,
                    )
                    at_t = apool.tile([128, 512], F16, tag="at", name="at_t")
                    nc.vector.tensor_mul(
                        at_t[:, 0:mw], pu[ci][:, 0:mw], rb_ps[:, 0:mw]
                    )
                    if subs_on_act:
                        nc.scalar.activation(
                            d_tiles[ci // 2][:, ci % 2, 0:mw], at_t[:, 0:mw],
                            AF.Identity, bias=neg1[:],
                        )
                    else:
                        nc.vector.tensor_scalar_sub(
                            d_tiles[ci // 2][:, ci % 2, 0:mw], at_t[:, 0:mw], 1.0
                        )
                return d_tiles

            def emit_tail(d_tiles, m0, mw, last=False, j_range=None):
                """Emit mm2 + store for a finished stripe (d8 tiles ready)."""
                for j in (j_range if j_range is not None else range(NC_)):
                    o_ps = pso.tile([128, 512], F32, tag="o", name="o_ps")
                    for c in range(KC2):
                        nc.tensor.matmul(
                            o_ps[:, 0:mw],
                            w2_k[c][:, :, j * 128:(j + 1) * 128],
                            d_tiles[c][:, :, 0:mw],
                            start=(c == 0),
                            stop=(c == KC2 - 1),
                            perf_mode=DR,
                        )
                    o_t = opool.tile([128, 512], F16, tag="ost", name="o_t")
                    # the very last output block is staged out in two halves
                    # so its store overlaps the ACT copy of the second half
                    halves = (
                        [(0, mw // 2), (mw // 2, mw)]
                        if (last and j == NC_ - 1) else [(0, mw)]
                    )
                    for lo, hi in halves:
                        nc.scalar.activation(
                            o_t[:, lo:hi], o_ps[:, lo:hi], AF.Identity,
                            bias=bias_t[:, j:j + 1], scale=1.0 / (WSCALE * WSCALE),
                        )
                        nc.sync.dma_start(
                            outT[j * 128:(j + 1) * 128, m0 + lo:m0 + hi],
                            o_t[:, lo:hi],
                        )

            for si, (m0, mw) in enumerate(STRIPES):
                if si == 0:
                    xt_k = xt0
                elif si in xpre:
                    xt_k = xpre[si]
                else:
                    xt_k = []
                    for c in range(KC2):
                        t = xpool.tile(
                            [128, 2, 512], F8, tag=f"xt_{c}", name=f"xt{si}_{c}"
                        )
                        nc.sync.dma_start(
                            t[:, :, 0:mw], xt_v[:, c, :, m0:m0 + mw]
                        )
                        xt_k.append(t)

                # ---- previous stripe normalization (rb block first: its d8
                # chain then hides under mm1 of this stripe) ----
                d_tiles = emit_norm(prev[0], prev[1], prev[3]) if prev else None
                last = si == len(STRIPES) - 1

                # ---- mm1: q-projection, contiguous 32-MM DoubleRow block ----
                u_tiles = []
                u8_tiles = [
                    u8pool.tile([128, 2, 512], F8, tag=f"u8_{c}", name=f"u8_{c}")
                    for c in range(KC2)
                ]

                def emit_mm1(ci_range):
                    for ci in ci_range:
                        # Borrow the sel-sum PSUM bank (idle during mm1) for
                        # every 4th q tile: the q ring is effectively 4 deep,
                        # so mm1[ci] c0 waits on exp(ci-4), not exp(ci-3).
                        if ci % 4 == 3:
                            q_ps = pss.tile([128, 512], F32, tag="s", name="q_ps_s")
                        else:
                            q_ps = psq.tile([128, 512], F32, tag="q", name="q_ps")
                        for c in range(KC2):
                            nc.tensor.matmul(
                                q_ps[:, 0:mw],
                                w1_k[c][:, :, ci * 128:(ci + 1) * 128],
                                xt_k[c][:, :, 0:mw],
                                start=(c == 0),
                                stop=(c == KC2 - 1),
                                perf_mode=DR,
                            )
                        u_t = upool.tile([128, 512], F16, tag="u", name="u_t")
                        nc.scalar.activation(
                            u_t[:, 0:mw], q_ps[:, 0:mw], AF.Exp, scale=0.25 / WSCALE
                        )
                        nc.vector.tensor_scalar_mul(
                            u8_tiles[ci // 2][:, ci % 2, 0:mw], u_t[:, 0:mw], 1.0
                        )
                        u_tiles.append(u_t)

                if si == 0:
                    # Stripe 0's x chunks land ~1.1us apart at the DMA
                    # roofline while the c-inner loop wants all four at once.
                    # For ci 0-3, run c as the OUTER loop across 4 open PSUM
                    # accumulation groups (q-ring x3 + s bank) so each x
                    # chunk is fully consumed before the next is needed.
                    qps0 = [
                        pss.tile([128, 512], F32, tag="s", name="q_ps_s0")
                        if ci == 3 else
                        psq.tile([128, 512], F32, tag="q", name=f"q_ps0_{ci}")
                        for ci in range(4)
                    ]
                    for c in range(KC2):
                        for ci in range(4):
                            nc.tensor.matmul(
                                qps0[ci][:, 0:mw],
                                w1_k[c][:, :, ci * 128:(ci + 1) * 128],
                                xt_k[c][:, :, 0:mw],
                                start=(c == 0),
                                stop=(c == KC2 - 1),
                                perf_mode=DR,
                            )
                    for ci in range(4):
                        u_t = upool.tile([128, 512], F16, tag="u", name="u_t")
                        nc.scalar.activation(
                            u_t[:, 0:mw], qps0[ci][:, 0:mw],
                            AF.Exp, scale=0.25 / WSCALE,
                        )
                        nc.vector.tensor_scalar_mul(
                            u8_tiles[ci // 2][:, ci % 2, 0:mw], u_t[:, 0:mw], 1.0
                        )
                        u_tiles.append(u_t)
                    emit_mm1(range(4, NC_))
                elif last and d_tiles is not None:
                    # Last stripe: there is no next stripe to hide this
                    # stripe's tail under, so pull the PREVIOUS stripe's mm2
                    # forward — half into the middle of mm1 (its output
                    # copies unblock while mm1's second half runs) and half
                    # after sel (absorbing the rb reciprocal-chain latency) —
                    # leaving only sel+rb+mm2(last) exposed after the final
                    # mm1 block.
                    emit_mm1(range(0, 4))
                    emit_tail(d_tiles, prev[2], prev[3], j_range=range(0, 4))
                    emit_mm1(range(4, NC_))
                else:
                    emit_mm1(range(NC_))

                # ---- head sums (4-MM fp8 DoubleRow block) + reciprocal ----
                s_ps = pss.tile([128, 512], F32, tag="s", name="s_ps")
                for c in range(KC2):
                    nc.tensor.matmul(
                        s_ps[0:HEADS, 0:mw],
                        sel8_t[:, c, :, :],
                        u8_tiles[c][:, :, 0:mw],
                        start=(c == 0),
                        stop=(c == KC2 - 1),
                        perf_mode=DR,
                    )
                # sel8 entries are 1/64, so s_ps = s/64 and rcp32 = 64/s
                rcp32 = spool.tile([HEADS, 512], F32, tag="rcp32", name="rcp32")
                nc.vector.reciprocal_approx_fast(rcp32[:, 0:mw], s_ps[0:HEADS, 0:mw])
                # 64/s as an fp8 hi/lo pair (so the rb broadcast matmul can be
                # DoubleRow like its neighbors - no fp16<->fp8 weight-buffer
                # mode switches on the PE).  Rows 16+ zeroed on GpSimd.
                rcp_t = spool.tile([128, 2, 512], F8, tag="rcp", name="rcp_t")
                nc.gpsimd.memset(rcp_t[:, :, 0:mw], 0.0)
                nc.scalar.activation(
                    rcp_t[0:HEADS, 0, 0:mw], rcp32[:, 0:mw], AF.Copy
                )
                nc.vector.scalar_tensor_tensor(
                    rcp_t[0:HEADS, 1, 0:mw],
                    rcp_t[0:HEADS, 0, 0:mw],
                    -1.0,
                    rcp32[:, 0:mw],
                    op0=mybir.AluOpType.mult,
                    op1=mybir.AluOpType.add,
                )

                if last:
                    # second half of the previous stripe's tail fills the
                    # rb(last) reciprocal-chain latency; then the last
                    # stripe's own norm block (rb + d8 chain, subs on the
                    # now-idle ACT engine).
                    if d_tiles is not None:
                        emit_tail(d_tiles, prev[2], prev[3], j_range=range(4, NC_))
                    d_last = emit_norm(u_tiles, rcp_t, mw, subs_on_act=True)
                else:
                    # ---- previous stripe output projection ----
                    if d_tiles is not None:
                        emit_tail(d_tiles, prev[2], prev[3])
                prev = (u_tiles, rcp_t, m0, mw)

            # epilogue: last stripe's output projection
            emit_tail(d_last, prev[2], prev[3], last=True)
    nc.compile()
    return nc


_NC_CACHE = None
LAST_RESULT = None


def _ensure_ntff_hook():
    """bass_utils' axon trace path needs antenv.axon_hooks, which this
    container's antenv lacks. Provide it + register the ctypes NTFF hook."""
    import types

    try:
        from antenv.axon_hooks import get_axon_ntff_profile_hook  # noqa: F401
        return True
    except ImportError:
        pass
    try:
        import antenv
        from trn_agent_boot.trn_boot import _ntff_profile_via_ctypes

        m = types.ModuleType("antenv.axon_hooks")
        state = {"hook": None}
        m.set_axon_ntff_profile_hook = lambda h: state.__setitem__("hook", h)
        m.get_axon_ntff_profile_hook = lambda: state["hook"]
        sys.modules["antenv.axon_hooks"] = m
        antenv.axon_hooks = m
        m.set_axon_ntff_profile_hook(
            _ntff_profile_via_ctypes("/opt/axon/libaxon_pjrt.so")
        )
        return True
    except Exception as e:  # pragma: no cover
        print(f"ntff hook injection failed: {e}")
        return False


def _selectors():
    # head index of global feature n is n // 64.
    # sel8: DoubleRow selector for the head-sum, paired like u8/d8 tiles:
    #   plane (c, i) covers feature chunk ci = 2c+i, i.e. heads 2*ci (parts
    #   0..63) and 2*ci+1 (parts 64..127).
    # Entries are 1/64 so the head-sum comes out pre-scaled: s_ps = s/64,
    # making reciprocal_approx_fast produce 64/s directly.
    sel8 = np.zeros((128, KC2, 2, HEADS), np.float32)
    for c in range(KC2):
        for i in range(2):
            ci = 2 * c + i
            sel8[:64, c, i, 2 * ci] = 1.0 / 64.0
            sel8[64:, c, i, 2 * ci + 1] = 1.0 / 64.0
    # selt: transposed selector for the rcp broadcast matmul.  Two identical
    # 0/1 planes (DoubleRow pairs): plane 0 multiplies rcp_hi, plane 1 the
    # fp8 residual rcp_lo; their PSUM sum reconstructs 64/s to ~1e-3.
    selt = np.zeros((128, 2, NC_, 128), np.float32)
    for ci in range(NC_):
        for i in range(2):
            selt[2 * ci, i, ci, :64] = 1.0
            selt[2 * ci + 1, i, ci, 64:] = 1.0
    return (
        np.ascontiguousarray(sel8.reshape(128, KC2 * 2 * HEADS)).astype(_NF8),
        np.ascontiguousarray(selt.reshape(128, 2 * NC_ * 128)).astype(_NF8),
    )


def kernel(x, W1, W2, heads, trace=False):
    global _NC_CACHE, LAST_RESULT
    x = np.asarray(x, dtype=np.float32)
    W1 = np.asarray(W1, dtype=np.float32)
    W2 = np.asarray(W2, dtype=np.float32)

    X = x.reshape(M_TOTAL, E)
    X8T = X.astype(_NF8).T  # [E, M_TOTAL] view
    w1t = np.ascontiguousarray(W1[:E, :].T * WSCALE).astype(_NF8)  # 64*W1q[n,k]^T
    w2t = np.ascontiguousarray(W2.T * WSCALE).astype(_NF8)         # 64*W2[j,n]^T
    sel8, selt = _selectors()
    # bias[p, j] = S_{j*128+p} / 64 with S_j = sum_n W2[j, n] (exact fp32)
    bias = np.ascontiguousarray(
        (W2.sum(axis=1) / WSCALE).reshape(NC_, 128).T
    ).astype(np.float32)

    in_maps = []
    for c in range(N_CORES):
        xt_c = np.ascontiguousarray(X8T[:, c * M_CORE:(c + 1) * M_CORE])
        in_maps.append(
            {"xt": xt_c, "w1t": w1t, "w2t": w2t, "sel8": sel8, "selt": selt,
             "bias": bias}
        )

    if _NC_CACHE is None:
        _NC_CACHE = build_nc()

    if trace:
        trace = _ensure_ntff_hook()

    res = run_bass_kernel_spmd(_NC_CACHE, in_maps, list(range(N_CORES)), trace=trace)
    LAST_RESULT = res

    OT = np.concatenate(
        [np.asarray(res.results[c]["outT"]).astype(np.float32) for c in range(N_CORES)],
        axis=1,
    )
    return np.ascontiguousarray(OT.T).reshape(B, S, E)
